# revision 1
# baseline (speedup 1.0000x reference)
"""Trainium2 Bass kernel for nn_CAC_42511586296007 (circular-mask max-pool descriptor).

Reference (per batch b, channel c):
  v = l2norm_c(max_hw(x)) + sum over 153 circular masks m of l2norm_c(max_hw(x*m))
Masks: center + per-quadrant rings/circles of integer radius on 28x28.

v2 design (bf16 datapath, measured-cost driven):
  - batch sharded 8 ways (4/core). Per batch:
    1. DMA x[b] -> xq [128c, 8t, 784] f32.
    2. Quadrant mirror: 4 transposing copies f32->bf16 into M[cell(225), q(4), t(8)]
       (3 on ScalarE, 1 on GpSimd); -inf edge slots memset once per M buffer.
    3. One ap_gather (272 idx, d=32) emits the ring-sorted cell stream.
    4. Two pairwise TT-max folds (bf16, 2x mode) quarter the stream.
    5. Grouped strided reduce_max per equal-size ring band -> seg[b, k80, t8]
       (t innermost: the [G,q,t,s] AP order runs at full 1x rate).
  - Phase 2 per half (2 batches): fold seg -> full max; relu -> vt rings/center;
    circles = in-place Hillis-Steele prefix max; squares (TT 2x); channel norms
    via PE (ones stationary, accumulate over ct); sqrt/recip/broadcast; final
    scale + reduce_sum over slots.
"""

import numpy as np

_B, _C, _HH, _WW = 32, 1024, 28, 28
_S = _HH * _WW
_NCORES = 8
_BL = _B // _NCORES       # 4 batches per core
_CT = _C // 128           # 8 channel tiles
_NSEG = 80                # 76 rings + center + outer + 2 pad(-inf)
_NSLOT = 154              # full + center + 76 rings + 76 circles
_NEG = -3.0e38
_QUADS = [(1, 1), (-1, 1), (1, -1), (-1, -1)]   # (sign_x, sign_y) ref order


def _build_tables():
    ij = np.arange(15)
    I, J = np.meshgrid(ij, ij, indexing="ij")
    RING = np.ceil(np.sqrt(I * I + J * J)).astype(int)

    cells = []          # padded cell-id list, bucket order: rings r=1..19, center, outer
    groups = []         # (off4, G, s) : folded-by-4 offset, #rings, folded cnt per ring
    off = 0             # offset in cells (pre-fold)
    run = None
    for r in range(1, 20):
        cc = [i * 15 + j for i in range(15) for j in range(15) if RING[i, j] == r]
        pad = (-len(cc)) % 4
        cc = cc + cc[:1] * pad
        s = len(cc) // 4
        if run is not None and run[2] == s:
            run[1] += 1
        else:
            if run is not None:
                groups.append(tuple(run))
            run = [off // 4, 1, s]
        cells.extend(cc)
        off += len(cc)
    groups.append(tuple(run))
    center_off4 = off // 4
    cells.extend([0, 0, 0, 0])
    off += 4
    cc = [i * 15 + j for i in range(15) for j in range(15) if RING[i, j] >= 20]
    cc = cc + cc[:1] * ((-len(cc)) % 4)
    outer_off4 = off // 4
    cells.extend(cc)
    off += len(cc)
    # pad total to multiple of 16 (gather idx table layout)
    cells = cells + cells[-1:] * ((-len(cells)) % 16)
    n = len(cells)
    a = np.asarray(cells, dtype=np.int16).reshape(n // 16, 16).T
    idx_w = np.ascontiguousarray(np.tile(a, (8, 1)))   # [128, n//16]
    return idx_w, n, groups, center_off4, outer_off4


_IDXW, _NIDX, _GROUPS, _COFF, _OOFF = _build_tables()
_NF2 = _NIDX // 4          # folded-by-4 stream blocks
_NC_CACHE = None


def _build_nc():
    import concourse.bacc as bacc
    import concourse.mybir as mybir
    from concourse.tile import TileContext

    f32 = mybir.dt.float32
    bf16 = mybir.dt.bfloat16
    i16 = mybir.dt.int16
    AX = mybir.AxisListType
    AF = mybir.ActivationFunctionType
    MAX = mybir.AluOpType.max
    MULT = mybir.AluOpType.mult
    ADD = mybir.AluOpType.add

    nc = bacc.Bacc("TRN2")
    xs = nc.dram_tensor("xs", [_BL, _C, _S], f32, kind="ExternalInput")
    idx_d = nc.dram_tensor("idxg", [128, _NIDX // 16], i16, kind="ExternalInput")
    out_d = nc.dram_tensor("out", [128, _BL * _CT], f32, kind="ExternalOutput")

    with TileContext(nc) as tc:
        with (
            tc.tile_pool(name="const", bufs=1) as cpool,
            tc.tile_pool(name="x", bufs=3) as xpool,
            tc.tile_pool(name="big", bufs=1) as bpool,
            tc.tile_pool(name="g", bufs=2) as gpool,
            tc.tile_pool(name="f1", bufs=2) as f1pool,
            tc.tile_pool(name="f2", bufs=2) as f2pool,
            tc.tile_pool(name="sm", bufs=2) as smpool,
            tc.tile_pool(name="psn", bufs=4, space="PSUM") as ppool_n,
            tc.tile_pool(name="pbc", bufs=4, space="PSUM") as ppool_b,
        ):
            idx_t = cpool.tile([128, _NIDX // 16], i16, tag="idx")
            nc.sync.dma_start(out=idx_t[:], in_=idx_d[:])
            ones_b = cpool.tile([128, 1], bf16, tag="ones_b")
            nc.vector.memset(ones_b[:], 1.0)
            ones1 = cpool.tile([1, 128], f32, tag="ones1")
            nc.vector.memset(ones1[:], 1.0)

            # persistent M buffers (bf16), -inf edge slots set once
            M_pers = [bpool.tile([128, 225 * 32], bf16, tag=f"M{k}", name=f"M{k}")
                      for k in range(3)]
            for k in range(3):
                Mv0 = M_pers[k][:].rearrange(
                    "p (i j q t) -> p i j q t", i=15, j=15, q=4)
                nc.vector.memset(Mv0[:, 14, :, 0:2, :], _NEG)
                nc.vector.memset(Mv0[:, :, 14, 0, :], _NEG)
                nc.vector.memset(Mv0[:, :, 14, 2, :], _NEG)

            seg_h = [bpool.tile([128, 2 * _NSEG * _CT], bf16, tag=f"seg{h}",
                                name=f"seg{h}") for h in range(2)]
            seg_hv = [t[:].rearrange("p (b k t) -> p b k t", b=2, k=_NSEG)
                      for t in seg_h]
            for h in range(2):
                nc.vector.memset(seg_hv[h][:, :, 78:80, :], _NEG)
            vt_h = [bpool.tile([128, 2 * _NSLOT * _CT], bf16, tag=f"vt{h}",
                               name=f"vt{h}") for h in range(2)]
            vt_hv = [t[:].rearrange("p (b k t) -> p b k t", b=2, k=_NSLOT)
                     for t in vt_h]
            outv = cpool.tile([128, _BL * _CT], f32, tag="outv")

            g_tiles = {}

            def do_mirror(b):
                xq = xpool.tile([128, _CT * _S], f32, tag="xq")
                if b == 0:
                    nc.sync.dma_start(
                        out=xq[:, 0:4 * _S],
                        in_=xs[b, 0:512].rearrange("(t p) s -> p t s", p=128))
                    nc.sync.dma_start(
                        out=xq[:, 4 * _S:],
                        in_=xs[b, 512:].rearrange("(t p) s -> p t s", p=128))
                else:
                    nc.sync.dma_start(
                        out=xq[:], in_=xs[b].rearrange("(t p) s -> p t s", p=128))
                xq_v = xq[:].rearrange("p (t a c) -> p t a c", t=_CT, a=_HH)

                M = M_pers[b % 3]
                M_v = M[:].rearrange("p (i j q t) -> p i j q t", i=15, j=15, q=4)
                for qi, (sx, sy) in enumerate(_QUADS):
                    ic = 14 if sy == 1 else 15
                    jc = 14 if sx == 1 else 15
                    src = xq_v[
                        :, :,
                        (slice(14, 14 + ic) if sy == 1 else slice(14, None, -1)),
                        (slice(14, 14 + jc) if sx == 1 else slice(14, None, -1)),
                    ].transpose([0, 2, 3, 1])
                    dst = M_v[:, 0:ic, 0:jc, qi, :]
                    if b == 0:
                        if qi < 2:
                            nc.vector.tensor_copy(out=dst, in_=src)
                        else:
                            nc.scalar.activation(out=dst, in_=src, func=AF.Copy)
                    elif qi == 2:
                        nc.scalar.activation(
                            out=dst[:, :, :, 0:4], in_=src[:, :, :, 0:4],
                            func=AF.Copy)
                        nc.vector.tensor_copy(
                            out=dst[:, :, :, 4:8], in_=src[:, :, :, 4:8])
                    else:
                        nc.scalar.activation(out=dst, in_=src, func=AF.Copy)

                g = gpool.tile([128, _NIDX * 32], bf16, tag="g")
                k1 = 144
                nc.gpsimd.ap_gather(
                    out_ap=g[:, 0:k1 * 32], in_ap=M[:],
                    idxs_ap=idx_t[:, 0:k1 // 16],
                    channels=128, num_elems=225, d=32, num_idxs=k1)
                nc.gpsimd.ap_gather(
                    out_ap=g[:, k1 * 32:], in_ap=M[:],
                    idxs_ap=idx_t[:, k1 // 16:],
                    channels=128, num_elems=225, d=32, num_idxs=_NIDX - k1)
                g_tiles[b] = g

            def do_compute(b):
                g = g_tiles[b]
                f1 = f1pool.tile([128, _NIDX * 16], bf16, tag="f1")
                gp = g[:].rearrange("p (s w) -> p s w", w=64)
                nc.vector.tensor_tensor(
                    out=f1[:].rearrange("p (s w) -> p s w", w=32),
                    in0=gp[:, :, 0:32], in1=gp[:, :, 32:64], op=MAX)
                f2 = f2pool.tile([128, _NF2 * 32], bf16, tag="f2")
                f1p = f1[:].rearrange("p (s w) -> p s w", w=64)
                nc.vector.tensor_tensor(
                    out=f2[:].rearrange("p (s w) -> p s w", w=32),
                    in0=f1p[:, :, 0:32], in1=f1p[:, :, 32:64], op=MAX)

                h, bl = b // 2, b % 2
                segv = seg_hv[h]
                r0 = 0
                for off4, G, s in _GROUPS:
                    nc.vector.reduce_max(
                        out=segv[:, bl, 1 + r0 * 4:1 + (r0 + G) * 4, :].rearrange(
                            "p (G q) t -> p G q t", q=4),
                        in_=f2[:, off4 * 32:(off4 + G * s) * 32].rearrange(
                            "p (G s q t) -> p G q t s", G=G, s=s, q=4),
                        axis=AX.X)
                    r0 += G
                nc.vector.reduce_max(
                    out=segv[:, bl, 0, :],
                    in_=f2[:, _COFF * 32:(_COFF + 1) * 32].rearrange(
                        "p (q t) -> p t q", q=4),
                    axis=AX.X)
                nc.vector.reduce_max(
                    out=segv[:, bl, 77, :],
                    in_=f2[:, _OOFF * 32:(_OOFF + 1) * 32].rearrange(
                        "p (q t) -> p t q", q=4),
                    axis=AX.X)

            def do_phase2(h, bl):
                segv = seg_hv[h][:, bl:bl + 1, :, :]
                vtv = vt_hv[h][:, bl:bl + 1, :, :]
                # full max: one strided reduce over all 80 seg slots
                nc.vector.reduce_max(
                    out=vtv[:, 0, 0, :],
                    in_=segv[:, 0, :, :].transpose([0, 2, 1]),
                    axis=AX.X)
                # relu: center + rings
                nc.vector.tensor_scalar_max(
                    vtv[:, :, 1:78, :], segv[:, :, 0:77, :], 0.0)
                # circles: init max(ring, center), then Hillis prefix (in place)
                nc.vector.tensor_tensor(
                    out=vtv[:, :, 78:154, :], in0=vtv[:, :, 2:78, :],
                    in1=vtv[:, :, 1:2, :].broadcast_to((128, 1, 76, _CT)),
                    op=MAX)
                for s in (1, 2, 4, 8, 16):
                    nc.vector.tensor_tensor(
                        out=vtv[:, :, 78 + 4 * s:154, :],
                        in0=vtv[:, :, 78 + 4 * s:154, :],
                        in1=vtv[:, :, 78:154 - 4 * s, :], op=MAX)
                # squares + channel-norm matmuls
                sq = smpool.tile([128, _NSLOT * _CT], bf16, tag="sq")
                nc.scalar.activation(
                    out=sq[:], in_=vtv.rearrange("p b k t -> p (b k t)"),
                    func=AF.Square)
                sq_v = sq[:].rearrange("p (k t) -> p k t", k=_NSLOT)
                nrm = smpool.tile([1, _NSLOT], f32, tag="nrm")
                inv = smpool.tile([1, _NSLOT], f32, tag="inv")
                scr1 = smpool.tile([1, _NSLOT], f32, tag="scr1")
                ps = ppool_n.tile([1, _NSLOT], f32, tag="psn")
                for ct in range(_CT):
                    nc.tensor.matmul(
                        ps[:], ones_b[:], sq_v[:, :, ct],
                        start=(ct == 0), stop=(ct == _CT - 1))
                nc.scalar.activation(out=nrm[:], in_=ps[:], func=AF.Sqrt)
                nc.vector.reciprocal_approx_fast(out=inv[:], in_=nrm[:])
                scr = smpool.tile([128, _NSLOT * _CT], bf16, tag="scr")
                pb = ppool_b.tile([128, _NSLOT], f32, tag="pbc")
                nc.tensor.matmul(pb[:], ones1[:], inv[:], start=True, stop=True)
                pbs = smpool.tile([128, _NSLOT], bf16, tag="pbs")
                nc.scalar.activation(out=pbs[:], in_=pb[:], func=AF.Copy)
                nc.vector.tensor_tensor(
                    out=scr[:].rearrange("p (k t) -> p k t", k=_NSLOT),
                    in0=vtv[:, 0, :, :],
                    in1=pbs[:][:, :, None].broadcast_to((128, _NSLOT, _CT)),
                    op=MULT)
                nc.vector.tensor_tensor(
                    out=scr[:, 0:77 * _CT], in0=scr[:, 0:77 * _CT],
                    in1=scr[:, 77 * _CT:154 * _CT], op=ADD)
                b = h * 2 + bl
                nc.vector.reduce_sum(
                    out=outv[:, b * _CT:(b + 1) * _CT],
                    in_=scr[:, 0:77 * _CT].rearrange("p (k t) -> p t k", k=77),
                    axis=AX.X)

            do_mirror(0)
            do_mirror(1)
            do_compute(0)
            do_mirror(2)
            do_phase2(0, 0)
            do_compute(1)
            do_mirror(3)
            do_phase2(0, 1)
            do_compute(2)
            do_phase2(1, 0)
            do_compute(3)
            do_phase2(1, 1)
            nc.sync.dma_start(out=out_d[:], in_=outv[:])

    nc.finalize()
    return nc


def _get_nc():
    global _NC_CACHE
    if _NC_CACHE is None:
        _NC_CACHE = _build_nc()
    return _NC_CACHE


def _run(x, trace=False):
    from concourse.bass_utils import run_bass_kernel_spmd

    nc = _get_nc()
    x = np.ascontiguousarray(np.asarray(x, dtype=np.float32))
    xs = x.reshape(_NCORES, _BL, _C, _S)
    in_maps = [
        {"xs": np.ascontiguousarray(xs[c]), "idxg": _IDXW}
        for c in range(_NCORES)
    ]
    res = run_bass_kernel_spmd(
        nc, in_maps, core_ids=list(range(_NCORES)), trace=trace)
    out = np.empty((_B, _C), np.float32)
    for c in range(_NCORES):
        r = np.asarray(res.results[c]["out"])            # [128, 32]
        rr = r.reshape(128, _BL, _CT)                    # [p, b, ct]
        out[c * _BL:(c + 1) * _BL] = rr.transpose(1, 2, 0).reshape(_BL, _C)
    return out.reshape(_B, _C, 1, 1), res


def kernel(x):
    out, _ = _run(x, trace=False)
    return out

